# revision 28
# baseline (speedup 1.0000x reference)
"""CondInst fused kernel for 8 Trainium2 NeuronCores.

The reference output depends only on batch element 0 of cnn_feature:
  - params are gathered from ctrl[0] at detection centers
  - feats is a broadcast of mask_feats[0]
so the tower/controller work for batches 1..3 is dead code, and the
controller conv is only needed at the 100 detection positions.

Strategy (embarrassingly parallel, no collectives):
  - Spatially shard batch-0 across the 8 cores: 20 output rows each,
    with a 4-row halo on the input so the 4 chained 3x3 convs need no
    inter-core exchange.  Image-boundary SAME-padding is enforced by
    per-core BN scale/shift vectors that are zeroed for out-of-image
    rows (relu(x*0+0) == 0).
  - The controller conv at the 100 detection points is a tiny matmul on
    host-gathered 3x3 patches (contract dim 1152), computed on-device.
  - The dynamic mask head runs on every core for all 100 instances over
    that core's 3200 pixels:
      layer0: stacked matmul, lhsT [10, 800] shared rhs (rel-coords are
              folded into per-instance biases; the grid term is shared)
      layer1: block-diagonal matmuls, 16 instances per 128x128 tile
      layer2: per-partition scalar multiply + block-ones matmul

Layout trick: the controller weight columns are host-permuted so every
on-device rearrangement of the dynamic params is a plain contiguous DMA:
  cols   0:80   w0 stored c'*8+o, c' ordered (feats 0..8, rel-x, rel-y)
  cols  80:144  w1 stored o*8+o'  (per-instance transposed)
  cols 144:152 w2, 152:160 b0, 160:168 b1, 168 b2 (unchanged)

Compute dtype: KERNEL_DT env = "bf16" (default, full-rate matmuls,
rel err ~1e-2) or "fp32" (native fp32 matmuls, 4 passes, slower).
"""

import os
import numpy as np

B, CIN, H, W = 4, 128, 160, 160
K = 100
CH = 8
OUT = 8
STRIDE = 4
EPS = 1e-5
NCORES = 8

ROWS = H // NCORES          # 20 output rows per core
F = W + 2                   # padded row width 162
HALO = 4
RIN = ROWS + 2 * HALO       # 28 input rows per core
P3 = ROWS * F               # 3240 padded pixels per core
NCHUNK = 486                # mask-head / proj free-dim chunk (3 rows)
CONTRACT = CIN * 9          # 1152
NPARAM = 185                # permuted dynamic-param vector width
GROUPS = [(g * 16, min(16, K - g * 16)) for g in range((K + 15) // 16)]

_CACHE = {}


def _mode():
    return os.environ.get("KERNEL_DT", "bf16")


def _param_perm():
    """new param index -> original param index (185,)

    cols 0:80    w0 stored c'*8+o, c' ordered (feats 0..8, rel-x, rel-y)
    cols 80:144  w1 stored o*8+o' (per-instance transposed)
    cols 144:184 fan block, col 144 + o*5 + q with q = (w0x, w0y, b0,
                 b1, w2) -- interleaved per channel so the on-device
                 per-group fan scatter is one rectangular 3D DMA.
                 (w0x/w0y duplicate cols 64:80, which l0 also reads.)
    col 184      b2
    """
    perm = np.zeros(NPARAM, np.int64)
    corder = [2, 3, 4, 5, 6, 7, 8, 9, 0, 1]
    for cp, c in enumerate(corder):
        for o in range(8):
            perm[cp * 8 + o] = o * 10 + c         # w0
    for o in range(8):
        for o2 in range(8):
            perm[80 + o * 8 + o2] = 80 + o2 * 8 + o   # w1 transposed per-instance
    for o in range(8):
        perm[144 + o * 5 + 0] = o * 10 + 0        # w0x
        perm[144 + o * 5 + 1] = o * 10 + 1        # w0y
        perm[144 + o * 5 + 2] = 152 + o           # b0
        perm[144 + o * 5 + 3] = 160 + o           # b1
        perm[144 + o * 5 + 4] = 144 + o           # w2
    perm[184] = 168                               # b2
    return perm


def _host_prep(inputs):
    """Build the 8 per-core input maps (pure numpy indexing + packing)."""
    import ml_dtypes
    cdt_np = np.float32 if _mode() == "fp32" else ml_dtypes.bfloat16

    cnn_feature = np.asarray(inputs["cnn_feature"], np.float32)
    tower_w = np.asarray(inputs["tower_w"], np.float32)
    bn_gamma = np.asarray(inputs["bn_gamma"], np.float32)
    bn_beta = np.asarray(inputs["bn_beta"], np.float32)
    bn_mean = np.asarray(inputs["bn_mean"], np.float32)
    bn_var = np.asarray(inputs["bn_var"], np.float32)
    proj_w = np.asarray(inputs["proj_w"], np.float32)
    proj_b = np.asarray(inputs["proj_b"], np.float32)
    ctrl_w = np.asarray(inputs["ctrl_w"], np.float32)
    ctrl_b = np.asarray(inputs["ctrl_b"], np.float32)
    detection = np.asarray(inputs["detection"])

    x0 = cnn_feature[0]                                   # [128, 160, 160]

    # tower weights as lhsT per tap: twT[i*9+ky*3+kx] = W[i,:,:,ky,kx].T
    twT = np.ascontiguousarray(
        tower_w.transpose(0, 3, 4, 2, 1).reshape(36, 128, 128)).astype(cdt_np)

    # BN scale/shift
    inv = bn_gamma / np.sqrt(bn_var + EPS)                # [4, 128]
    shift = bn_beta - bn_mean * inv                       # [4, 128]

    # controller weights, column-permuted, +bias row, padded to 1280 contract
    perm = _param_perm()
    cw_flat = ctrl_w.reshape(169, CONTRACT)
    cwT = np.zeros((1280, NPARAM), np.float32)
    cwT[:CONTRACT, :] = cw_flat[perm].T
    cwT[CONTRACT, :] = ctrl_b[perm]

    # patches at detection centers, transposed, +ones row; fused with cwT
    # into one tensor so each 128-contract chunk is a single DMA (the fp32
    # matmul codegen allows only one sync wait on its weight-load slot)
    xs = detection[:, 0].astype(np.int64)
    ys = detection[:, 1].astype(np.int64)
    xpad2 = np.pad(x0, ((0, 0), (1, 1), (1, 1)))
    pcw = np.zeros((1280, K + NPARAM), np.float32)
    for k in range(K):
        pcw[:CONTRACT, k] = xpad2[:, ys[k]:ys[k] + 3, xs[k]:xs[k] + 3].ravel()
    pcw[CONTRACT, :K] = 1.0
    pcw[:, K:] = cwT

    # detection centers replicated 8x along partitions, per 16-instance group:
    # detfan[kl*8+o, g] = 4*x_{16g+kl} (cols 0..7), 4*y (cols 7..14)
    det4 = detection.astype(np.float32) * STRIDE
    detfan = np.zeros((128, 14), np.float32)
    for g, (k0, gsz) in enumerate(GROUPS):
        for kl in range(gsz):
            detfan[kl * 8:kl * 8 + 8, g] = det4[k0 + kl, 0]
            detfan[kl * 8:kl * 8 + 8, 7 + g] = det4[k0 + kl, 1]

    onesbd = np.zeros((128, 16), np.float32)
    for kl in range(16):
        onesbd[kl * 8:kl * 8 + 8, kl] = 1.0

    projT = np.ascontiguousarray(proj_w.T).astype(cdt_np)  # [128, 8]
    projb = proj_b.reshape(8, 1).astype(np.float32)

    # per-core padded input slices
    xpad_rows = np.zeros((128, H + 2 * HALO, F), np.float32)
    xpad_rows[:, HALO:HALO + H, 1:161] = x0
    xpad_rows = xpad_rows.astype(cdt_np)

    shared = dict(twT=twT, pcw=pcw.astype(cdt_np), detfan=detfan,
                  onesbd=onesbd.astype(cdt_np), projT=projT, projb=projb)

    in_maps = []
    for c in range(NCORES):
        xin = np.ascontiguousarray(xpad_rows[:, ROWS * c:ROWS * c + RIN, :])

        # bnv[ch, i, region, 0/1] = inv/shift; zeroed for out-of-image regions
        bnv = np.zeros((128, 4, 3, 2), np.float32)
        for i in range(4):
            bnv[:, i, 1, 0] = inv[i]
            bnv[:, i, 1, 1] = shift[i]
            if c != 0:
                bnv[:, i, 0, 0] = inv[i]
                bnv[:, i, 0, 1] = shift[i]
            if c != NCORES - 1:
                bnv[:, i, 2, 0] = inv[i]
                bnv[:, i, 2, 1] = shift[i]

        grid = np.zeros((2, ROWS, F), np.float32)
        gxrow = -(np.arange(W, dtype=np.float32) * STRIDE + STRIDE // 2)
        gyv = -(np.arange(ROWS * c, ROWS * c + ROWS, dtype=np.float32) * STRIDE
                + STRIDE // 2)
        grid[0, :, 1:161] = gxrow[None, :]
        grid[1, :, 1:161] = gyv[:, None]

        in_maps.append(dict(shared, xin=xin,
                            bnv=bnv.reshape(128, 24),
                            grid=grid.reshape(2, ROWS * F).astype(cdt_np)))
    return in_maps

def _build_program(reps=1):
    """Restructured vs the first working version:

    - weight-assembly scatter is ~23 batched multi-dim DMAs (was ~160
      tiny ones costing ~70us of HWDGE/SWDGE issue): the block-diagonal
      bd matrix is filled with 16 kl-indexed DMAs whose (group, o, o')
      dims are all pure partition- or column-strides.
    - conv4 + proj + dynamic head are interleaved per 3-row chunk, with
      conv4 running one chunk ahead and the head's L0/L1/L2 matmuls
      software-pipelined (skew 2) so PE never waits on a relu.
    - the 7 per-group L2 results land in one [112, nn] PSUM tile
      (disjoint 16-partition ranges) so the +b2 bias is ONE op per chunk
      instead of 7 strided 16-partition ops.
    - elementwise is split: Activation = conv BN + proj bias + half the
      relus; DVE = other half + the L2 bias-add + phase-B assembly math.
    """
    from contextlib import ExitStack
    import concourse.bass as bass
    import concourse.tile as tile
    from concourse import bacc, mybir

    f32 = mybir.dt.float32
    cdt = f32 if _mode() == "fp32" else mybir.dt.bfloat16
    Relu = mybir.ActivationFunctionType.Relu
    Ident = mybir.ActivationFunctionType.Identity
    Alu = mybir.AluOpType

    def man_ap(base, rel_off, dims):
        """manual flat-element AP: dims = [[stride, count], ...]"""
        return bass.AP(tensor=base.tensor, offset=base.offset + rel_off,
                       ap=[list(d) for d in dims])

    nc = bacc.Bacc("TRN2", target_bir_lowering=False, debug=False,
                   enable_asserts=False, detect_race_conditions=False)

    xin_d = nc.dram_tensor("xin", [128, RIN, F], cdt, kind="ExternalInput")
    twT_d = nc.dram_tensor("twT", [36, 128, 128], cdt, kind="ExternalInput")
    bnv_d = nc.dram_tensor("bnv", [128, 24], f32, kind="ExternalInput")
    grid_d = nc.dram_tensor("grid", [2, P3], cdt, kind="ExternalInput")
    pcw_d = nc.dram_tensor("pcw", [1280, K + NPARAM], cdt, kind="ExternalInput")
    detfan_d = nc.dram_tensor("detfan", [128, 14], f32, kind="ExternalInput")
    onesbd_d = nc.dram_tensor("onesbd", [128, 16], cdt, kind="ExternalInput")
    projT_d = nc.dram_tensor("projT", [128, 8], cdt, kind="ExternalInput")
    projb_d = nc.dram_tensor("projb", [8, 1], f32, kind="ExternalInput")
    out_d = nc.dram_tensor("out", [K, ROWS, W], f32, kind="ExternalOutput")
    p2d_d = nc.dram_tensor("p2d", [112, NPARAM], cdt, kind="Internal")

    NC2 = K + NPARAM   # pcw row width (285)

    with tile.TileContext(nc) as tc, ExitStack() as octx:
      for rep in range(reps):
       with ExitStack() as ctx:
        const = ctx.enter_context(tc.tile_pool(name=f"const{rep}", bufs=1))
        prep = ctx.enter_context(tc.tile_pool(name=f"prep{rep}", bufs=1))
        convp = ctx.enter_context(tc.tile_pool(name=f"conv{rep}", bufs=1))

        # ---------- input DMAs (merged to minimize issue cost) ----------
        # first xin/twT chunk ahead of pcw: conv1's first matmuls only need
        # input rows 0..7 and taps 0..8, so the tower can start ~immediately
        # while pcw (for the params matmuls) streams in behind
        pm = ctx.enter_context(tc.tile_pool(name=f"pm{rep}", bufs=1))
        pc_all = pm.tile([128, 10 * NC2], cdt)
        xbuf = convp.tile([128, RIN * F + 2], cdt, tag="xbuf")
        nc.vector.memset(xbuf[:, 0:1], 0.0)
        nc.vector.memset(xbuf[:, 1 + RIN * F:], 0.0)
        tw_all = const.tile([128, 36 * 128], cdt)
        tw_sb = [tw_all[:, t * 128:(t + 1) * 128] for t in range(36)]

        def _xin_rows(r0, r1):
            nc.sync.dma_start(out=xbuf[:, 1 + r0 * F:1 + r1 * F],
                              in_=xin_d[:, r0:r1, :])

        def _tw_dma(t0, t1):
            nc.sync.dma_start(
                out=man_ap(tw_all[:], t0 * 128,
                           [[36 * 128, 128], [128, t1 - t0], [1, 128]]),
                in_=man_ap(twT_d[:], t0 * 128 * 128,
                           [[128, 128], [128 * 128, t1 - t0], [1, 128]]))

        # startup criticality order: conv chunk 0 needs rows 0..5 + tap 0
        # first, so those land ~1.5us before the rest
        _xin_rows(0, 5)
        _tw_dma(0, 1)
        _tw_dma(1, 9)
        _xin_rows(5, 14)
        for h in range(5):
            nc.sync.dma_start(
                out=man_ap(pc_all[:], 2 * h * NC2,
                           [[10 * NC2, 128], [NC2, 2], [1, NC2]]),
                in_=man_ap(pcw_d[:], 2 * h * 128 * NC2,
                           [[NC2, 128], [128 * NC2, 2], [1, NC2]]))
        _xin_rows(14, 21)
        _xin_rows(21, 28)
        for h in range(1, 4):
            _tw_dma(9 * h, 9 * (h + 1))

        bnv_sb = const.tile([128, 24], f32)
        nc.sync.dma_start(out=bnv_sb[:], in_=bnv_d[:])
        hbase = const.tile([10, P3], cdt)
        nc.sync.dma_start(out=hbase[8:10, :], in_=grid_d[:])
        detfan_sb = const.tile([128, 14], f32)
        nc.sync.dma_start(out=detfan_sb[:], in_=detfan_d[:])
        onesbd_sb = const.tile([128, 16], cdt)
        nc.sync.dma_start(out=onesbd_sb[:], in_=onesbd_d[:])
        projT_sb = const.tile([128, 8], cdt)
        nc.sync.dma_start(out=projT_sb[:], in_=projT_d[:])
        projb_sb = const.tile([8, 1], f32)
        nc.sync.dma_start(out=projb_sb[:], in_=projb_d[:])

        # ---------- phase C start: conv layer 1, first two chunks ----------
        # (emitted before the params matmuls so conv1 isn't gated on the
        # pcw DMAs in the in-order PE queue; params land during chunk 1)
        conv_ps = ctx.enter_context(
            tc.tile_pool(name=f"conv_ps{rep}", bufs=2, space="PSUM"))

        # p-state warmup: ~6 dummy matmuls on a zeroed tile keep PE busy
        # from ~0.8us so it reaches full clock before the first real conv
        # matmul (input DMAs land ~4us); their psum slots recycle via the
        # same cps ring the conv uses
        warm = convp.tile([128, 512], cdt, tag="warm")
        nc.vector.memset(warm[:], 0.0)
        for _ in range(6):
            wp = conv_ps.tile([128, 512], f32, tag="cps")
            nc.tensor.matmul(wp[:], lhsT=warm[:, 0:128], rhs=warm[:],
                             start=True, stop=True)

        def _conv_chunk(i, cur, rout, r0, obuf3):
            nr = min(3, rout - r0)
            ps = conv_ps.tile([128, nr * F], f32, tag="cps")
            for t, (ky, kx) in enumerate(
                    (ky, kx) for ky in range(3) for kx in range(3)):
                off = 1 + (r0 + ky) * F + kx - 1
                nc.tensor.matmul(
                    ps[:], lhsT=tw_sb[i * 9 + t],
                    rhs=cur[:, off:off + nr * F],
                    start=(t == 0), stop=(t == 8))
            ps3 = ps[:].rearrange("p (r c) -> p r c", c=F)
            T = 3 - i  # out-of-image candidate rows at top/bottom
            bounds = sorted({0, T, rout - T, rout})
            for rs, re in zip(bounds[:-1], bounds[1:]):
                a, b = max(rs, r0), min(re, r0 + nr)
                if a >= b:
                    continue
                reg = 0 if b <= T else (2 if a >= rout - T else 1)
                sidx = (i * 3 + reg) * 2
                nc.scalar.activation(
                    out=obuf3[:, a:b, 1:161],
                    in_=ps3[:, a - r0:b - r0, 1:161],
                    func=Relu,
                    scale=bnv_sb[:, sidx:sidx + 1],
                    bias=bnv_sb[:, sidx + 1:sidx + 2])

        def _mk_obuf(i, rout):
            obuf = convp.tile([128, rout * F + 2], cdt, tag=f"c{i}")
            obuf3 = obuf[:, 1:1 + rout * F].rearrange("p (r c) -> p r c", c=F)
            nc.vector.memset(obuf[:, 0:1], 0.0)
            nc.vector.memset(obuf[:, 1 + rout * F:], 0.0)
            nc.vector.memset(obuf3[:, :, 0:1], 0.0)
            nc.vector.memset(obuf3[:, :, 161:162], 0.0)
            return obuf, obuf3

        obuf0, obuf03 = _mk_obuf(0, RIN - 2)
        for r0 in range(0, 24, 3):
            _conv_chunk(0, xbuf, RIN - 2, r0, obuf03)

        # ---------- phase A: dynamic params P2[k, j'] = patches @ ctrl -----
        with tc.tile_pool(name=f"pm_ps{rep}", bufs=1, space="PSUM") as pm_ps:
            p2p = pm_ps.tile([K, NPARAM], f32)
            for i in range(10):
                nc.tensor.matmul(p2p[:],
                                 lhsT=pc_all[:, i * NC2:i * NC2 + K],
                                 rhs=pc_all[:, i * NC2 + K:(i + 1) * NC2],
                                 start=(i == 0), stop=(i == 9))
            # 112 rows: 12 zero rows pad group 6 to a rectangular 7x16
            # instance grid so every assembly DMA below is one rectangle
            p2 = prep.tile([112, NPARAM], cdt)
            nc.vector.memset(p2[:], 0.0)
            nc.vector.tensor_copy(p2[0:100, :], p2p[:])
        P2W = NPARAM

        # ---------- phase B: head weight assembly (batched DMAs + DVE) ----
        # The scatters need multi-partition-strided reads of p2, which the
        # SBUF DMA path can't express (partition steps must lead and be
        # single-step) -- bounce p2 through a DRAM scratch instead: DRAM
        # APs are unconstrained, and every scatter OUT side below leads
        # with its partition dim.
        nc.sync.dma_start(out=p2d_d[:], in_=p2[:])

        # lhsT0 [10, 800]: l0[c, k*8+o] = P2[k, c*8+o] -- one 3D DMA
        l0 = prep.tile([10, 8 * K], cdt)
        nc.sync.dma_start(
            out=man_ap(l0[:], 0, [[8 * K, 10], [8, K], [1, 8]]),
            in_=man_ap(p2d_d[:], 0, [[8, 10], [P2W, K], [1, 8]]))

        # block-diagonal L1 weights, all groups in one [128, 896] tile
        # (group g at cols g*128): one DMA per kl covers all 7 groups,
        # traversal (o, g, o') with the partition dim leading on the out
        BDW = 7 * 128
        bdall = prep.tile([128, BDW], cdt)
        nc.gpsimd.memset(bdall[:], 0.0)
        for kl in range(16):
            nc.sync.dma_start(
                out=man_ap(bdall[:], kl * 8 * BDW + kl * 8,
                           [[BDW, 8], [128, 7], [1, 8]]),
                in_=man_ap(p2d_d[:], kl * P2W + 80,
                           [[8, 8], [16 * P2W, 7], [1, 8]]))

        # fan-out block [kl*8+o, g*5+q], q = (w0x, w0y, b0, b1, w2): the
        # interleaved p2 cols 144 + o*5 + q make each kl's fan one 2D DMA
        # (out rows kl*8..kl*8+8 get all 35 (g,q) cols contiguously; zero
        # rows of p2 fill the fake instances)
        FW = 35
        fanstage = prep.tile([128, FW], cdt)
        for kl in range(16):
            nc.sync.dma_start(
                out=man_ap(fanstage[:], kl * 8 * FW,
                           [[FW, 8], [5, 7], [1, 5]]),
                in_=man_ap(p2d_d[:], kl * P2W + 144,
                           [[5, 8], [16 * P2W, 7], [1, 5]]))
        # b2 per instance as a [112, 1] column (partition == instance id)
        b2stage = prep.tile([112, 1], cdt)
        nc.sync.dma_start(
            out=man_ap(b2stage[:], 0, [[1, 112]]),
            in_=man_ap(p2d_d[:], 184, [[P2W, 112]]))

        fans = prep.tile([128, FW], f32)
        nc.vector.tensor_copy(fans[:], fanstage[:])
        # strided [128, 7] views, one col per group for each quantity q
        w0xfan, w0yfan, b0fan, b1fan, w2fan = (
            man_ap(fans[:], q, [[FW, 128], [5, 7]]) for q in range(5))
        b2col = prep.tile([112, 1], f32)
        nc.vector.tensor_copy(b2col[:], b2stage[:])

        # beta0fan = b0 + w0x*4x + w0y*4y  (per-instance bias, layer 0)
        beta0fan = prep.tile([128, 7], f32)
        tmpf = prep.tile([128, 7], f32)
        nc.vector.tensor_mul(beta0fan[:], w0xfan, detfan_sb[:, 0:7])
        nc.vector.tensor_mul(tmpf[:], w0yfan, detfan_sb[:, 7:14])
        nc.vector.tensor_add(beta0fan[:], beta0fan[:], tmpf[:])
        nc.vector.tensor_add(beta0fan[:], beta0fan[:], b0fan)

        # layer-2 weights: [gp, 112] per group, nonzero only in its own 16
        # output columns -- the 7 L2 matmuls then accumulate into ONE
        # [112, nn] psum tile (PE out base-partition must be 0/32/64, so
        # disjoint 16-row output slices are not addressable directly)
        bd2 = []
        for g, (k0, gsz) in enumerate(GROUPS):
            gp = gsz * 8
            bdw2 = prep.tile([gp, 112], cdt, tag=f"bdw2_{g}")
            nc.vector.memset(bdw2[:], 0.0)
            nc.vector.tensor_scalar_mul(bdw2[:, 16 * g:16 * g + 16],
                                        onesbd_sb[0:gp, :],
                                        fans[0:gp, g * 5 + 4:g * 5 + 5])
            bd2.append(bdw2)

        # ---------- phase C rest: conv layers 1..3 ----------
        for r0 in range(24, RIN - 2, 3):
            _conv_chunk(0, xbuf, RIN - 2, r0, obuf03)
        cur = obuf0
        rcur = RIN - 2
        for i in range(1, 3):
            rout = rcur - 2
            obuf, obuf3 = _mk_obuf(i, rout)
            for r0 in range(0, rout, 3):
                _conv_chunk(i, cur, rout, r0, obuf3)
            cur = obuf
            rcur = rout

        # ---------- phase D: conv4 + proj + dynamic head, interleaved ----
        # conv4 runs one chunk ahead of the head; the head's 21 matmuls per
        # chunk are software-pipelined (L1 two groups behind L0, L2 two
        # behind L1) so relus on Act/DVE complete before PE consumes them.
        NCH = (ROWS + 2) // 3          # 7 chunks of <=3 rows
        def _relu_bias(eng, out, in_, bias):
            if eng == "act":
                nc.scalar.activation(out=out, in_=in_, func=Relu, bias=bias)
            else:
                nc.vector.tensor_scalar(out=out, in0=in_, scalar1=bias,
                                        scalar2=0.0, op0=Alu.add, op1=Alu.max)

        with tc.tile_pool(name=f"c4p{rep}", bufs=2) as c4p, \
             tc.tile_pool(name=f"head{rep}", bufs=6) as headp, \
             tc.tile_pool(name=f"outp{rep}", bufs=2) as outp, \
             tc.tile_pool(name=f"pp_ps{rep}", bufs=1, space="PSUM") as pp_ps, \
             tc.tile_pool(name=f"hps0{rep}", bufs=2, space="PSUM") as hps0, \
             tc.tile_pool(name=f"hps1{rep}", bufs=2, space="PSUM") as hps1, \
             tc.tile_pool(name=f"hps2{rep}", bufs=1, space="PSUM") as hps2:

            def _conv4_thunks(ci):
                """12 emission thunks for conv4+BN+proj of chunk ci, to be
                interleaved between the previous head chunk's t-steps so
                PE never waits on a relu."""
                r0 = 3 * ci
                nr = min(3, ROWS - r0)
                nn = nr * F
                n0 = r0 * F
                state = {}

                def _mm(t):
                    def run():
                        if t == 0:
                            cps = conv_ps.tile([128, nn], f32, tag="cps")
                            state["cps"] = cps
                        ky, kx = t // 3, t % 3
                        off = 1 + (r0 + ky) * F + kx - 1
                        nc.tensor.matmul(
                            state["cps"][:], lhsT=tw_sb[27 + t],
                            rhs=cur[:, off:off + nn],
                            start=(t == 0), stop=(t == 8))
                    return run

                def _bn():
                    # layer-4 rows are all in-image: single mid-region
                    # BN+relu (pad cols get junk, masked at out DMA)
                    c4 = c4p.tile([128, nn], cdt, tag="c4")
                    state["c4"] = c4
                    nc.scalar.activation(out=c4[:], in_=state["cps"][:],
                                         func=Relu,
                                         scale=bnv_sb[:, 20:21],
                                         bias=bnv_sb[:, 21:22])

                def _proj():
                    pp = pp_ps.tile([8, nn], f32, tag="pps")
                    nc.tensor.matmul(pp[:], lhsT=projT_sb[:],
                                     rhs=state["c4"][:],
                                     start=True, stop=True)
                    nc.scalar.activation(out=hbase[0:8, n0:n0 + nn],
                                         in_=pp[:], func=Ident,
                                         bias=projb_sb[:, 0:1])

                return [_mm(t) for t in range(9)] + [_bn, _proj]

            def _head_chunk(ci, fill):
                r0 = 3 * ci
                nr = min(3, ROWS - r0)
                nn = nr * F
                n0 = r0 * F
                ps2 = hps2.tile([112, nn], f32, tag="ps2")
                fi = 0
                h1cs, h2cs = {}, {}
                for t in range(11):
                    if fi < len(fill):
                        fill[fi]()
                        fi += 1
                    if t < 7:
                        g = t
                        k0, gsz = GROUPS[g]
                        gp = gsz * 8
                        ps0 = hps0.tile([gp, nn], f32, tag="ps0")
                        nc.tensor.matmul(ps0[:],
                                         lhsT=l0[:, 8 * k0:8 * k0 + gp],
                                         rhs=hbase[:, n0:n0 + nn],
                                         start=True, stop=True)
                        h1c = headp.tile([gp, nn], cdt, tag="h1c")
                        _relu_bias("act" if g % 2 == 0 else "dve",
                                   h1c[:], ps0[:], beta0fan[0:gp, g:g + 1])
                        h1cs[g] = h1c
                    if 2 <= t < 9:
                        g = t - 2
                        k0, gsz = GROUPS[g]
                        gp = gsz * 8
                        ps1 = hps1.tile([gp, nn], f32, tag="ps1")
                        nc.tensor.matmul(ps1[:],
                                         lhsT=bdall[0:gp,
                                                    g * 128:g * 128 + gp],
                                         rhs=h1cs[g][:],
                                         start=True, stop=True)
                        h2c = headp.tile([gp, nn], cdt, tag="h2c")
                        _relu_bias("dve" if g % 2 == 0 else "act",
                                   h2c[:], ps1[:],
                                   fans[0:gp, g * 5 + 3:g * 5 + 4])
                        h2cs[g] = h2c
                    if 4 <= t:
                        g = t - 4
                        k0, gsz = GROUPS[g]
                        gp = gsz * 8
                        nc.tensor.matmul(ps2[:], lhsT=bd2[g][:],
                                         rhs=h2cs[g][:],
                                         start=(g == 0), stop=(g == 6))
                while fi < len(fill):
                    fill[fi]()
                    fi += 1
                outg = outp.tile([112, nn], f32, tag="outg")
                nc.vector.tensor_scalar(out=outg[:], in0=ps2[:],
                                        scalar1=b2col[:, 0:1], scalar2=None,
                                        op0=Alu.add)
                ogv = outg[0:K, :].rearrange("p (r c) -> p r c", c=F)
                nc.sync.dma_start(out=out_d[:, r0:r0 + nr, :],
                                  in_=ogv[:, :, 1:161])

            for f in _conv4_thunks(0):
                f()
            for ci in range(1, NCH + 1):
                fill = _conv4_thunks(ci) if ci < NCH else []
                _head_chunk(ci - 1, fill)
    nc.compile()
    return nc



def _get_program(reps=1):
    key = (_mode(), reps)
    if key not in _CACHE:
        _CACHE[key] = _build_program(reps)
    return _CACHE[key]


def _run(in_maps, trace=False, reps=1, **kwargs):
    from concourse.bass_utils import run_bass_kernel_spmd
    nc = _get_program(reps)
    return run_bass_kernel_spmd(nc, in_maps, core_ids=list(range(NCORES)),
                                trace=trace, **kwargs)


def kernel(**inputs) -> np.ndarray:
    in_maps = _host_prep(inputs)
    res = _run(in_maps)
    out = np.concatenate([res.results[c]["out"] for c in range(NCORES)], axis=1)
    return out.astype(np.float32)



# revision 30
# speedup vs baseline: 1.0239x; 1.0239x over previous
"""CondInst fused kernel for 8 Trainium2 NeuronCores.

The reference output depends only on batch element 0 of cnn_feature:
  - params are gathered from ctrl[0] at detection centers
  - feats is a broadcast of mask_feats[0]
so the tower/controller work for batches 1..3 is dead code, and the
controller conv is only needed at the 100 detection positions.

Strategy (embarrassingly parallel, no collectives):
  - Spatially shard batch-0 across the 8 cores: 20 output rows each,
    with a 4-row halo on the input so the 4 chained 3x3 convs need no
    inter-core exchange.  Image-boundary SAME-padding is enforced by
    per-core BN scale/shift vectors that are zeroed for out-of-image
    rows (relu(x*0+0) == 0).
  - The controller conv at the 100 detection points is a tiny matmul on
    host-gathered 3x3 patches (contract dim 1152), computed on-device.
  - The dynamic mask head runs on every core for all 100 instances over
    that core's 3200 pixels:
      layer0: stacked matmul, lhsT [10, 800] shared rhs (rel-coords are
              folded into per-instance biases; the grid term is shared)
      layer1: block-diagonal matmuls, 16 instances per 128x128 tile
      layer2: per-partition scalar multiply + block-ones matmul

Layout trick: the controller weight columns are host-permuted so every
on-device rearrangement of the dynamic params is a plain contiguous DMA:
  cols   0:80   w0 stored c'*8+o, c' ordered (feats 0..8, rel-x, rel-y)
  cols  80:144  w1 stored o*8+o'  (per-instance transposed)
  cols 144:152 w2, 152:160 b0, 160:168 b1, 168 b2 (unchanged)

Compute dtype: KERNEL_DT env = "bf16" (default, full-rate matmuls,
rel err ~1e-2) or "fp32" (native fp32 matmuls, 4 passes, slower).
"""

import os
import numpy as np

B, CIN, H, W = 4, 128, 160, 160
K = 100
CH = 8
OUT = 8
STRIDE = 4
EPS = 1e-5
NCORES = 8

ROWS = H // NCORES          # 20 output rows per core
F = W + 2                   # padded row width 162
HALO = 4
RIN = ROWS + 2 * HALO       # 28 input rows per core
P3 = ROWS * F               # 3240 padded pixels per core
NCHUNK = 486                # mask-head / proj free-dim chunk (3 rows)
CONTRACT = CIN * 9          # 1152
NPARAM = 185                # permuted dynamic-param vector width
GROUPS = [(g * 16, min(16, K - g * 16)) for g in range((K + 15) // 16)]

_CACHE = {}


def _mode():
    return os.environ.get("KERNEL_DT", "bf16")


def _param_perm():
    """new param index -> original param index (185,)

    cols 0:80    w0 stored c'*8+o, c' ordered (feats 0..8, rel-x, rel-y)
    cols 80:144  w1 stored o*8+o' (per-instance transposed)
    cols 144:184 fan block, col 144 + o*5 + q with q = (w0x, w0y, b0,
                 b1, w2) -- interleaved per channel so the on-device
                 per-group fan scatter is one rectangular 3D DMA.
                 (w0x/w0y duplicate cols 64:80, which l0 also reads.)
    col 184      b2
    """
    perm = np.zeros(NPARAM, np.int64)
    corder = [2, 3, 4, 5, 6, 7, 8, 9, 0, 1]
    for cp, c in enumerate(corder):
        for o in range(8):
            perm[cp * 8 + o] = o * 10 + c         # w0
    for o in range(8):
        for o2 in range(8):
            perm[80 + o * 8 + o2] = 80 + o2 * 8 + o   # w1 transposed per-instance
    for o in range(8):
        perm[144 + o * 5 + 0] = o * 10 + 0        # w0x
        perm[144 + o * 5 + 1] = o * 10 + 1        # w0y
        perm[144 + o * 5 + 2] = 152 + o           # b0
        perm[144 + o * 5 + 3] = 160 + o           # b1
        perm[144 + o * 5 + 4] = 144 + o           # w2
    perm[184] = 168                               # b2
    return perm


def _host_prep(inputs):
    """Build the 8 per-core input maps (pure numpy indexing + packing)."""
    import ml_dtypes
    cdt_np = np.float32 if _mode() == "fp32" else ml_dtypes.bfloat16

    cnn_feature = np.asarray(inputs["cnn_feature"], np.float32)
    tower_w = np.asarray(inputs["tower_w"], np.float32)
    bn_gamma = np.asarray(inputs["bn_gamma"], np.float32)
    bn_beta = np.asarray(inputs["bn_beta"], np.float32)
    bn_mean = np.asarray(inputs["bn_mean"], np.float32)
    bn_var = np.asarray(inputs["bn_var"], np.float32)
    proj_w = np.asarray(inputs["proj_w"], np.float32)
    proj_b = np.asarray(inputs["proj_b"], np.float32)
    ctrl_w = np.asarray(inputs["ctrl_w"], np.float32)
    ctrl_b = np.asarray(inputs["ctrl_b"], np.float32)
    detection = np.asarray(inputs["detection"])

    x0 = cnn_feature[0]                                   # [128, 160, 160]

    # tower weights as lhsT per tap: tap t = i*9+ky*3+kx holds
    # W[i,:,:,ky,kx].T; stored PARTITION-major [cin, t*128+cout] so each
    # tap-range load is one fully contiguous DMA (512B+ descriptor runs
    # avoid the DMA read-modify-write penalty)
    twT = np.ascontiguousarray(
        tower_w.transpose(0, 3, 4, 2, 1).reshape(36, 128, 128)
        .transpose(1, 0, 2).reshape(128, 36 * 128)).astype(cdt_np)

    # BN scale/shift
    inv = bn_gamma / np.sqrt(bn_var + EPS)                # [4, 128]
    shift = bn_beta - bn_mean * inv                       # [4, 128]

    # controller weights, column-permuted, +bias row, padded to 1280 contract
    perm = _param_perm()
    cw_flat = ctrl_w.reshape(169, CONTRACT)
    cwT = np.zeros((1280, NPARAM), np.float32)
    cwT[:CONTRACT, :] = cw_flat[perm].T
    cwT[CONTRACT, :] = ctrl_b[perm]

    # patches at detection centers, transposed, +ones row; fused with cwT
    # into one tensor so each 128-contract chunk is a single DMA (the fp32
    # matmul codegen allows only one sync wait on its weight-load slot)
    xs = detection[:, 0].astype(np.int64)
    ys = detection[:, 1].astype(np.int64)
    xpad2 = np.pad(x0, ((0, 0), (1, 1), (1, 1)))
    pcw = np.zeros((1280, K + NPARAM), np.float32)
    for k in range(K):
        pcw[:CONTRACT, k] = xpad2[:, ys[k]:ys[k] + 3, xs[k]:xs[k] + 3].ravel()
    pcw[CONTRACT, :K] = 1.0
    pcw[:, K:] = cwT

    # detection centers replicated 8x along partitions, per 16-instance group:
    # detfan[kl*8+o, g] = 4*x_{16g+kl} (cols 0..7), 4*y (cols 7..14)
    det4 = detection.astype(np.float32) * STRIDE
    detfan = np.zeros((128, 14), np.float32)
    for g, (k0, gsz) in enumerate(GROUPS):
        for kl in range(gsz):
            detfan[kl * 8:kl * 8 + 8, g] = det4[k0 + kl, 0]
            detfan[kl * 8:kl * 8 + 8, 7 + g] = det4[k0 + kl, 1]

    onesbd = np.zeros((128, 16), np.float32)
    for kl in range(16):
        onesbd[kl * 8:kl * 8 + 8, kl] = 1.0

    projT = np.ascontiguousarray(proj_w.T).astype(cdt_np)  # [128, 8]
    projb = proj_b.reshape(8, 1).astype(np.float32)

    # per-core padded input slices
    xpad_rows = np.zeros((128, H + 2 * HALO, F), np.float32)
    xpad_rows[:, HALO:HALO + H, 1:161] = x0
    xpad_rows = xpad_rows.astype(cdt_np)

    shared = dict(twT=twT, pcw=pcw.astype(cdt_np), detfan=detfan,
                  onesbd=onesbd.astype(cdt_np), projT=projT, projb=projb)

    in_maps = []
    for c in range(NCORES):
        xin = np.ascontiguousarray(xpad_rows[:, ROWS * c:ROWS * c + RIN, :])

        # bnv[ch, i, region, 0/1] = inv/shift; zeroed for out-of-image regions
        bnv = np.zeros((128, 4, 3, 2), np.float32)
        for i in range(4):
            bnv[:, i, 1, 0] = inv[i]
            bnv[:, i, 1, 1] = shift[i]
            if c != 0:
                bnv[:, i, 0, 0] = inv[i]
                bnv[:, i, 0, 1] = shift[i]
            if c != NCORES - 1:
                bnv[:, i, 2, 0] = inv[i]
                bnv[:, i, 2, 1] = shift[i]

        grid = np.zeros((2, ROWS, F), np.float32)
        gxrow = -(np.arange(W, dtype=np.float32) * STRIDE + STRIDE // 2)
        gyv = -(np.arange(ROWS * c, ROWS * c + ROWS, dtype=np.float32) * STRIDE
                + STRIDE // 2)
        grid[0, :, 1:161] = gxrow[None, :]
        grid[1, :, 1:161] = gyv[:, None]

        in_maps.append(dict(shared, xin=xin,
                            bnv=bnv.reshape(128, 24),
                            grid=grid.reshape(2, ROWS * F).astype(cdt_np)))
    return in_maps

def _build_program(reps=1):
    """Restructured vs the first working version:

    - weight-assembly scatter is ~23 batched multi-dim DMAs (was ~160
      tiny ones costing ~70us of HWDGE/SWDGE issue): the block-diagonal
      bd matrix is filled with 16 kl-indexed DMAs whose (group, o, o')
      dims are all pure partition- or column-strides.
    - conv4 + proj + dynamic head are interleaved per 3-row chunk, with
      conv4 running one chunk ahead and the head's L0/L1/L2 matmuls
      software-pipelined (skew 2) so PE never waits on a relu.
    - the 7 per-group L2 results land in one [112, nn] PSUM tile
      (disjoint 16-partition ranges) so the +b2 bias is ONE op per chunk
      instead of 7 strided 16-partition ops.
    - elementwise is split: Activation = conv BN + proj bias + half the
      relus; DVE = other half + the L2 bias-add + phase-B assembly math.
    """
    from contextlib import ExitStack
    import concourse.bass as bass
    import concourse.tile as tile
    from concourse import bacc, mybir

    f32 = mybir.dt.float32
    cdt = f32 if _mode() == "fp32" else mybir.dt.bfloat16
    Relu = mybir.ActivationFunctionType.Relu
    Ident = mybir.ActivationFunctionType.Identity
    Alu = mybir.AluOpType

    def man_ap(base, rel_off, dims):
        """manual flat-element AP: dims = [[stride, count], ...]"""
        return bass.AP(tensor=base.tensor, offset=base.offset + rel_off,
                       ap=[list(d) for d in dims])

    nc = bacc.Bacc("TRN2", target_bir_lowering=False, debug=False,
                   enable_asserts=False, detect_race_conditions=False)

    xin_d = nc.dram_tensor("xin", [128, RIN, F], cdt, kind="ExternalInput")
    twT_d = nc.dram_tensor("twT", [128, 36 * 128], cdt, kind="ExternalInput")
    bnv_d = nc.dram_tensor("bnv", [128, 24], f32, kind="ExternalInput")
    grid_d = nc.dram_tensor("grid", [2, P3], cdt, kind="ExternalInput")
    pcw_d = nc.dram_tensor("pcw", [1280, K + NPARAM], cdt, kind="ExternalInput")
    detfan_d = nc.dram_tensor("detfan", [128, 14], f32, kind="ExternalInput")
    onesbd_d = nc.dram_tensor("onesbd", [128, 16], cdt, kind="ExternalInput")
    projT_d = nc.dram_tensor("projT", [128, 8], cdt, kind="ExternalInput")
    projb_d = nc.dram_tensor("projb", [8, 1], f32, kind="ExternalInput")
    out_d = nc.dram_tensor("out", [K, ROWS, W], f32, kind="ExternalOutput")
    p2d_d = nc.dram_tensor("p2d", [112, NPARAM], cdt, kind="Internal")

    NC2 = K + NPARAM   # pcw row width (285)

    with tile.TileContext(nc) as tc, ExitStack() as octx:
      for rep in range(reps):
       with ExitStack() as ctx:
        const = ctx.enter_context(tc.tile_pool(name=f"const{rep}", bufs=1))
        prep = ctx.enter_context(tc.tile_pool(name=f"prep{rep}", bufs=1))
        convp = ctx.enter_context(tc.tile_pool(name=f"conv{rep}", bufs=1))

        # ---------- input DMAs (merged to minimize issue cost) ----------
        # first xin/twT chunk ahead of pcw: conv1's first matmuls only need
        # input rows 0..7 and taps 0..8, so the tower can start ~immediately
        # while pcw (for the params matmuls) streams in behind
        pm = ctx.enter_context(tc.tile_pool(name=f"pm{rep}", bufs=1))
        pc_all = pm.tile([128, 10 * NC2], cdt)
        xbuf = convp.tile([128, RIN * F + 2], cdt, tag="xbuf")
        nc.vector.memset(xbuf[:, 0:1], 0.0)
        nc.vector.memset(xbuf[:, 1 + RIN * F:], 0.0)
        tw_all = const.tile([128, 36 * 128], cdt)
        tw_sb = [tw_all[:, t * 128:(t + 1) * 128] for t in range(36)]

        def _xin_rows(r0, r1):
            nc.sync.dma_start(out=xbuf[:, 1 + r0 * F:1 + r1 * F],
                              in_=xin_d[:, r0:r1, :])

        def _tw_dma(t0, t1):
            nc.sync.dma_start(out=tw_all[:, t0 * 128:t1 * 128],
                              in_=twT_d[:, t0 * 128:t1 * 128])

        # startup criticality order: conv chunk 0 needs rows 0..5 + tap 0
        # first, so those land ~1.5us before the rest
        _xin_rows(0, 5)
        _tw_dma(0, 1)
        _tw_dma(1, 9)
        _xin_rows(5, 14)
        _xin_rows(14, 21)
        _xin_rows(21, 28)
        for h in range(5):
            nc.sync.dma_start(
                out=man_ap(pc_all[:], 2 * h * NC2,
                           [[10 * NC2, 128], [NC2, 2], [1, NC2]]),
                in_=man_ap(pcw_d[:], 2 * h * 128 * NC2,
                           [[NC2, 128], [128 * NC2, 2], [1, NC2]]))
        _tw_dma(9, 36)

        bnv_sb = const.tile([128, 24], f32)
        nc.sync.dma_start(out=bnv_sb[:], in_=bnv_d[:])
        hbase = const.tile([10, P3], cdt)
        nc.sync.dma_start(out=hbase[8:10, :], in_=grid_d[:])
        detfan_sb = const.tile([128, 14], f32)
        nc.sync.dma_start(out=detfan_sb[:], in_=detfan_d[:])
        onesbd_sb = const.tile([128, 16], cdt)
        nc.sync.dma_start(out=onesbd_sb[:], in_=onesbd_d[:])
        projT_sb = const.tile([128, 8], cdt)
        nc.sync.dma_start(out=projT_sb[:], in_=projT_d[:])
        projb_sb = const.tile([8, 1], f32)
        nc.sync.dma_start(out=projb_sb[:], in_=projb_d[:])

        # ---------- phase C start: conv layer 1, first two chunks ----------
        # (emitted before the params matmuls so conv1 isn't gated on the
        # pcw DMAs in the in-order PE queue; params land during chunk 1)
        conv_ps = ctx.enter_context(
            tc.tile_pool(name=f"conv_ps{rep}", bufs=2, space="PSUM"))

        # p-state warmup: ~6 dummy matmuls on a zeroed tile keep PE busy
        # from ~0.8us so it reaches full clock before the first real conv
        # matmul (input DMAs land ~4us); their psum slots recycle via the
        # same cps ring the conv uses
        warm = convp.tile([128, 512], cdt, tag="warm")
        nc.vector.memset(warm[:], 0.0)
        for _ in range(8):
            wp = conv_ps.tile([128, 512], f32, tag="cps")
            nc.tensor.matmul(wp[:], lhsT=warm[:, 0:128], rhs=warm[:],
                             start=True, stop=True)

        def _conv_chunk(i, cur, rout, r0, obuf3):
            nr = min(3, rout - r0)
            ps = conv_ps.tile([128, nr * F], f32, tag="cps")
            for t, (ky, kx) in enumerate(
                    (ky, kx) for ky in range(3) for kx in range(3)):
                off = 1 + (r0 + ky) * F + kx - 1
                nc.tensor.matmul(
                    ps[:], lhsT=tw_sb[i * 9 + t],
                    rhs=cur[:, off:off + nr * F],
                    start=(t == 0), stop=(t == 8))
            ps3 = ps[:].rearrange("p (r c) -> p r c", c=F)
            T = 3 - i  # out-of-image candidate rows at top/bottom
            bounds = sorted({0, T, rout - T, rout})
            for rs, re in zip(bounds[:-1], bounds[1:]):
                a, b = max(rs, r0), min(re, r0 + nr)
                if a >= b:
                    continue
                reg = 0 if b <= T else (2 if a >= rout - T else 1)
                sidx = (i * 3 + reg) * 2
                nc.scalar.activation(
                    out=obuf3[:, a:b, 1:161],
                    in_=ps3[:, a - r0:b - r0, 1:161],
                    func=Relu,
                    scale=bnv_sb[:, sidx:sidx + 1],
                    bias=bnv_sb[:, sidx + 1:sidx + 2])

        def _mk_obuf(i, rout):
            obuf = convp.tile([128, rout * F + 2], cdt, tag=f"c{i}")
            obuf3 = obuf[:, 1:1 + rout * F].rearrange("p (r c) -> p r c", c=F)
            nc.vector.memset(obuf[:, 0:1], 0.0)
            nc.vector.memset(obuf[:, 1 + rout * F:], 0.0)
            nc.vector.memset(obuf3[:, :, 0:1], 0.0)
            nc.vector.memset(obuf3[:, :, 161:162], 0.0)
            return obuf, obuf3

        obuf0, obuf03 = _mk_obuf(0, RIN - 2)
        for r0 in range(0, 24, 3):
            _conv_chunk(0, xbuf, RIN - 2, r0, obuf03)

        # ---------- phase A: dynamic params P2[k, j'] = patches @ ctrl -----
        with tc.tile_pool(name=f"pm_ps{rep}", bufs=1, space="PSUM") as pm_ps:
            p2p = pm_ps.tile([K, NPARAM], f32)
            for i in range(10):
                nc.tensor.matmul(p2p[:],
                                 lhsT=pc_all[:, i * NC2:i * NC2 + K],
                                 rhs=pc_all[:, i * NC2 + K:(i + 1) * NC2],
                                 start=(i == 0), stop=(i == 9))
            # 112 rows: 12 zero rows pad group 6 to a rectangular 7x16
            # instance grid so every assembly DMA below is one rectangle
            p2 = prep.tile([112, NPARAM], cdt)
            nc.vector.memset(p2[:], 0.0)
            nc.vector.tensor_copy(p2[0:100, :], p2p[:])
        P2W = NPARAM

        # ---------- phase B: head weight assembly (batched DMAs + DVE) ----
        # The scatters need multi-partition-strided reads of p2, which the
        # SBUF DMA path can't express (partition steps must lead and be
        # single-step) -- bounce p2 through a DRAM scratch instead: DRAM
        # APs are unconstrained, and every scatter OUT side below leads
        # with its partition dim.
        nc.sync.dma_start(out=p2d_d[:], in_=p2[:])

        # lhsT0 [10, 800]: l0[c, k*8+o] = P2[k, c*8+o] -- one 3D DMA
        l0 = prep.tile([10, 8 * K], cdt)
        nc.sync.dma_start(
            out=man_ap(l0[:], 0, [[8 * K, 10], [8, K], [1, 8]]),
            in_=man_ap(p2d_d[:], 0, [[8, 10], [P2W, K], [1, 8]]))

        # block-diagonal L1 weights, all groups in one [128, 896] tile
        # (group g at cols g*128): one DMA per kl covers all 7 groups,
        # traversal (o, g, o') with the partition dim leading on the out
        BDW = 7 * 128
        bdall = prep.tile([128, BDW], cdt)
        nc.gpsimd.memset(bdall[:], 0.0)
        for kl in range(16):
            nc.sync.dma_start(
                out=man_ap(bdall[:], kl * 8 * BDW + kl * 8,
                           [[BDW, 8], [128, 7], [1, 8]]),
                in_=man_ap(p2d_d[:], kl * P2W + 80,
                           [[8, 8], [16 * P2W, 7], [1, 8]]))

        # fan-out block [kl*8+o, g*5+q], q = (w0x, w0y, b0, b1, w2): the
        # interleaved p2 cols 144 + o*5 + q make each kl's fan one 2D DMA
        # (out rows kl*8..kl*8+8 get all 35 (g,q) cols contiguously; zero
        # rows of p2 fill the fake instances)
        FW = 35
        fanstage = prep.tile([128, FW], cdt)
        for kl in range(16):
            nc.sync.dma_start(
                out=man_ap(fanstage[:], kl * 8 * FW,
                           [[FW, 8], [5, 7], [1, 5]]),
                in_=man_ap(p2d_d[:], kl * P2W + 144,
                           [[5, 8], [16 * P2W, 7], [1, 5]]))
        # b2 per instance as a [112, 1] column (partition == instance id)
        b2stage = prep.tile([112, 1], cdt)
        nc.sync.dma_start(
            out=man_ap(b2stage[:], 0, [[1, 112]]),
            in_=man_ap(p2d_d[:], 184, [[P2W, 112]]))

        fans = prep.tile([128, FW], f32)
        nc.vector.tensor_copy(fans[:], fanstage[:])
        # strided [128, 7] views, one col per group for each quantity q
        w0xfan, w0yfan, b0fan, b1fan, w2fan = (
            man_ap(fans[:], q, [[FW, 128], [5, 7]]) for q in range(5))
        b2col = prep.tile([112, 1], f32)
        nc.vector.tensor_copy(b2col[:], b2stage[:])

        # beta0fan = b0 + w0x*4x + w0y*4y  (per-instance bias, layer 0)
        beta0fan = prep.tile([128, 7], f32)
        tmpf = prep.tile([128, 7], f32)
        nc.vector.tensor_mul(beta0fan[:], w0xfan, detfan_sb[:, 0:7])
        nc.vector.tensor_mul(tmpf[:], w0yfan, detfan_sb[:, 7:14])
        nc.vector.tensor_add(beta0fan[:], beta0fan[:], tmpf[:])
        nc.vector.tensor_add(beta0fan[:], beta0fan[:], b0fan)

        # layer-2 weights: [gp, 112] per group, nonzero only in its own 16
        # output columns -- the 7 L2 matmuls then accumulate into ONE
        # [112, nn] psum tile (PE out base-partition must be 0/32/64, so
        # disjoint 16-row output slices are not addressable directly)
        bd2 = []
        for g, (k0, gsz) in enumerate(GROUPS):
            gp = gsz * 8
            bdw2 = prep.tile([gp, 112], cdt, tag=f"bdw2_{g}")
            nc.vector.memset(bdw2[:], 0.0)
            nc.vector.tensor_scalar_mul(bdw2[:, 16 * g:16 * g + 16],
                                        onesbd_sb[0:gp, :],
                                        fans[0:gp, g * 5 + 4:g * 5 + 5])
            bd2.append(bdw2)

        # ---------- phase C rest: conv layers 1..3 ----------
        for r0 in range(24, RIN - 2, 3):
            _conv_chunk(0, xbuf, RIN - 2, r0, obuf03)
        cur = obuf0
        rcur = RIN - 2
        for i in range(1, 3):
            rout = rcur - 2
            obuf, obuf3 = _mk_obuf(i, rout)
            for r0 in range(0, rout, 3):
                _conv_chunk(i, cur, rout, r0, obuf3)
            cur = obuf
            rcur = rout

        # ---------- phase D: conv4 + proj + dynamic head, interleaved ----
        # conv4 runs one chunk ahead of the head; the head's 21 matmuls per
        # chunk are software-pipelined (L1 two groups behind L0, L2 two
        # behind L1) so relus on Act/DVE complete before PE consumes them.
        NCH = (ROWS + 2) // 3          # 7 chunks of <=3 rows
        def _relu_bias(eng, out, in_, bias):
            if eng == "act":
                nc.scalar.activation(out=out, in_=in_, func=Relu, bias=bias)
            else:
                nc.vector.tensor_scalar(out=out, in0=in_, scalar1=bias,
                                        scalar2=0.0, op0=Alu.add, op1=Alu.max)

        with tc.tile_pool(name=f"c4p{rep}", bufs=2) as c4p, \
             tc.tile_pool(name=f"head{rep}", bufs=6) as headp, \
             tc.tile_pool(name=f"outp{rep}", bufs=2) as outp, \
             tc.tile_pool(name=f"pp_ps{rep}", bufs=1, space="PSUM") as pp_ps, \
             tc.tile_pool(name=f"hps0{rep}", bufs=2, space="PSUM") as hps0, \
             tc.tile_pool(name=f"hps1{rep}", bufs=2, space="PSUM") as hps1, \
             tc.tile_pool(name=f"hps2{rep}", bufs=1, space="PSUM") as hps2:

            def _conv4_thunks(ci):
                """12 emission thunks for conv4+BN+proj of chunk ci, to be
                interleaved between the previous head chunk's t-steps so
                PE never waits on a relu."""
                r0 = 3 * ci
                nr = min(3, ROWS - r0)
                nn = nr * F
                n0 = r0 * F
                state = {}

                def _mm(t):
                    def run():
                        if t == 0:
                            cps = conv_ps.tile([128, nn], f32, tag="cps")
                            state["cps"] = cps
                        ky, kx = t // 3, t % 3
                        off = 1 + (r0 + ky) * F + kx - 1
                        nc.tensor.matmul(
                            state["cps"][:], lhsT=tw_sb[27 + t],
                            rhs=cur[:, off:off + nn],
                            start=(t == 0), stop=(t == 8))
                    return run

                def _bn():
                    # layer-4 rows are all in-image: single mid-region
                    # BN+relu (pad cols get junk, masked at out DMA)
                    c4 = c4p.tile([128, nn], cdt, tag="c4")
                    state["c4"] = c4
                    nc.scalar.activation(out=c4[:], in_=state["cps"][:],
                                         func=Relu,
                                         scale=bnv_sb[:, 20:21],
                                         bias=bnv_sb[:, 21:22])

                def _proj():
                    pp = pp_ps.tile([8, nn], f32, tag="pps")
                    nc.tensor.matmul(pp[:], lhsT=projT_sb[:],
                                     rhs=state["c4"][:],
                                     start=True, stop=True)
                    nc.scalar.activation(out=hbase[0:8, n0:n0 + nn],
                                         in_=pp[:], func=Ident,
                                         bias=projb_sb[:, 0:1])

                return [_mm(t) for t in range(9)] + [_bn, _proj]

            def _head_chunk(ci, fill):
                r0 = 3 * ci
                nr = min(3, ROWS - r0)
                nn = nr * F
                n0 = r0 * F
                ps2 = hps2.tile([112, nn], f32, tag="ps2")
                fi = 0
                h1cs, h2cs = {}, {}
                for t in range(11):
                    if fi < len(fill):
                        fill[fi]()
                        fi += 1
                    if t < 7:
                        g = t
                        k0, gsz = GROUPS[g]
                        gp = gsz * 8
                        ps0 = hps0.tile([gp, nn], f32, tag="ps0")
                        nc.tensor.matmul(ps0[:],
                                         lhsT=l0[:, 8 * k0:8 * k0 + gp],
                                         rhs=hbase[:, n0:n0 + nn],
                                         start=True, stop=True)
                        h1c = headp.tile([gp, nn], cdt, tag="h1c")
                        _relu_bias("act" if g % 2 == 0 else "dve",
                                   h1c[:], ps0[:], beta0fan[0:gp, g:g + 1])
                        h1cs[g] = h1c
                    if 2 <= t < 9:
                        g = t - 2
                        k0, gsz = GROUPS[g]
                        gp = gsz * 8
                        ps1 = hps1.tile([gp, nn], f32, tag="ps1")
                        nc.tensor.matmul(ps1[:],
                                         lhsT=bdall[0:gp,
                                                    g * 128:g * 128 + gp],
                                         rhs=h1cs[g][:],
                                         start=True, stop=True)
                        h2c = headp.tile([gp, nn], cdt, tag="h2c")
                        _relu_bias("dve" if g % 2 == 0 else "act",
                                   h2c[:], ps1[:],
                                   fans[0:gp, g * 5 + 3:g * 5 + 4])
                        h2cs[g] = h2c
                    if 4 <= t:
                        g = t - 4
                        k0, gsz = GROUPS[g]
                        gp = gsz * 8
                        nc.tensor.matmul(ps2[:], lhsT=bd2[g][:],
                                         rhs=h2cs[g][:],
                                         start=(g == 0), stop=(g == 6))
                while fi < len(fill):
                    fill[fi]()
                    fi += 1
                outg = outp.tile([112, nn], f32, tag="outg")
                nc.vector.tensor_scalar(out=outg[:], in0=ps2[:],
                                        scalar1=b2col[:, 0:1], scalar2=None,
                                        op0=Alu.add)
                ogv = outg[0:K, :].rearrange("p (r c) -> p r c", c=F)
                nc.sync.dma_start(out=out_d[:, r0:r0 + nr, :],
                                  in_=ogv[:, :, 1:161])

            for f in _conv4_thunks(0):
                f()
            for ci in range(1, NCH + 1):
                fill = _conv4_thunks(ci) if ci < NCH else []
                _head_chunk(ci - 1, fill)
    nc.compile()
    return nc



def _get_program(reps=1):
    key = (_mode(), reps)
    if key not in _CACHE:
        _CACHE[key] = _build_program(reps)
    return _CACHE[key]


def _run(in_maps, trace=False, reps=1, **kwargs):
    from concourse.bass_utils import run_bass_kernel_spmd
    nc = _get_program(reps)
    return run_bass_kernel_spmd(nc, in_maps, core_ids=list(range(NCORES)),
                                trace=trace, **kwargs)


def kernel(**inputs) -> np.ndarray:
    in_maps = _host_prep(inputs)
    res = _run(in_maps)
    out = np.concatenate([res.results[c]["out"] for c in range(NCORES)], axis=1)
    return out.astype(np.float32)



# revision 32
# speedup vs baseline: 1.1415x; 1.1149x over previous
"""CondInst fused kernel for 8 Trainium2 NeuronCores.

The reference output depends only on batch element 0 of cnn_feature:
  - params are gathered from ctrl[0] at detection centers
  - feats is a broadcast of mask_feats[0]
so the tower/controller work for batches 1..3 is dead code, and the
controller conv is only needed at the 100 detection positions.

Strategy (embarrassingly parallel, no collectives):
  - Spatially shard batch-0 across the 8 cores: 20 output rows each,
    with a 4-row halo on the input so the 4 chained 3x3 convs need no
    inter-core exchange.  Image-boundary SAME-padding is enforced by
    per-core BN scale/shift vectors that are zeroed for out-of-image
    rows (relu(x*0+0) == 0).
  - The controller conv at the 100 detection points is a tiny matmul on
    host-gathered 3x3 patches (contract dim 1152), computed on-device.
  - The dynamic mask head runs on every core for all 100 instances over
    that core's 3200 pixels:
      layer0: stacked matmul, lhsT [10, 800] shared rhs (rel-coords are
              folded into per-instance biases; the grid term is shared)
      layer1: block-diagonal matmuls, 16 instances per 128x128 tile
      layer2: per-partition scalar multiply + block-ones matmul

Layout trick: the controller weight columns are host-permuted so every
on-device rearrangement of the dynamic params is a plain contiguous DMA:
  cols   0:80   w0 stored c'*8+o, c' ordered (feats 0..8, rel-x, rel-y)
  cols  80:144  w1 stored o*8+o'  (per-instance transposed)
  cols 144:152 w2, 152:160 b0, 160:168 b1, 168 b2 (unchanged)

Compute dtype: KERNEL_DT env = "bf16" (default, full-rate matmuls,
rel err ~1e-2) or "fp32" (native fp32 matmuls, 4 passes, slower).
"""

import os
import numpy as np

B, CIN, H, W = 4, 128, 160, 160
K = 100
CH = 8
OUT = 8
STRIDE = 4
EPS = 1e-5
NCORES = 8

ROWS = H // NCORES          # 20 output rows per core
F = W + 2                   # padded row width 162
HALO = 4
RIN = ROWS + 2 * HALO       # 28 input rows per core
P3 = ROWS * F               # 3240 padded pixels per core
NCHUNK = 486                # mask-head / proj free-dim chunk (3 rows)
CONTRACT = CIN * 9          # 1152
NPARAM = 185                # permuted dynamic-param vector width
GROUPS = [(g * 16, min(16, K - g * 16)) for g in range((K + 15) // 16)]

_CACHE = {}


def _mode():
    return os.environ.get("KERNEL_DT", "bf16")


def _param_perm():
    """new param index -> original param index (185,)

    cols 0:80    w0 stored c'*8+o, c' ordered (feats 0..8, rel-x, rel-y)
    cols 80:144  w1 stored o*8+o' (per-instance transposed)
    cols 144:184 fan block, col 144 + o*5 + q with q = (w0x, w0y, b0,
                 b1, w2) -- interleaved per channel so the on-device
                 per-group fan scatter is one rectangular 3D DMA.
                 (w0x/w0y duplicate cols 64:80, which l0 also reads.)
    col 184      b2
    """
    perm = np.zeros(NPARAM, np.int64)
    corder = [2, 3, 4, 5, 6, 7, 8, 9, 0, 1]
    for cp, c in enumerate(corder):
        for o in range(8):
            perm[cp * 8 + o] = o * 10 + c         # w0
    for o in range(8):
        for o2 in range(8):
            perm[80 + o * 8 + o2] = 80 + o2 * 8 + o   # w1 transposed per-instance
    for o in range(8):
        perm[144 + o * 5 + 0] = o * 10 + 0        # w0x
        perm[144 + o * 5 + 1] = o * 10 + 1        # w0y
        perm[144 + o * 5 + 2] = 152 + o           # b0
        perm[144 + o * 5 + 3] = 160 + o           # b1
        perm[144 + o * 5 + 4] = 144 + o           # w2
    perm[184] = 168                               # b2
    return perm


def _host_prep(inputs):
    """Build the 8 per-core input maps (pure numpy indexing + packing)."""
    import ml_dtypes
    cdt_np = np.float32 if _mode() == "fp32" else ml_dtypes.bfloat16

    cnn_feature = np.asarray(inputs["cnn_feature"], np.float32)
    tower_w = np.asarray(inputs["tower_w"], np.float32)
    bn_gamma = np.asarray(inputs["bn_gamma"], np.float32)
    bn_beta = np.asarray(inputs["bn_beta"], np.float32)
    bn_mean = np.asarray(inputs["bn_mean"], np.float32)
    bn_var = np.asarray(inputs["bn_var"], np.float32)
    proj_w = np.asarray(inputs["proj_w"], np.float32)
    proj_b = np.asarray(inputs["proj_b"], np.float32)
    ctrl_w = np.asarray(inputs["ctrl_w"], np.float32)
    ctrl_b = np.asarray(inputs["ctrl_b"], np.float32)
    detection = np.asarray(inputs["detection"])

    x0 = cnn_feature[0]                                   # [128, 160, 160]

    # tower weights as lhsT per tap: tap t = i*9+ky*3+kx holds
    # W[i,:,:,ky,kx].T; stored PARTITION-major [cin, t*128+cout] so each
    # tap-range load is one fully contiguous DMA (512B+ descriptor runs
    # avoid the DMA read-modify-write penalty)
    twT = np.ascontiguousarray(
        tower_w.transpose(0, 3, 4, 2, 1).reshape(36, 128, 128)
        .transpose(1, 0, 2).reshape(128, 36 * 128)).astype(cdt_np)

    # BN scale/shift
    inv = bn_gamma / np.sqrt(bn_var + EPS)                # [4, 128]
    shift = bn_beta - bn_mean * inv                       # [4, 128]

    # controller weights, column-permuted, +bias row, padded to 1280 contract
    perm = _param_perm()
    cw_flat = ctrl_w.reshape(169, CONTRACT)
    cwT = np.zeros((1280, NPARAM), np.float32)
    cwT[:CONTRACT, :] = cw_flat[perm].T
    cwT[CONTRACT, :] = ctrl_b[perm]

    # patches at detection centers, transposed, +ones row; fused with cwT
    # into one tensor so each 128-contract chunk is a single DMA (the fp32
    # matmul codegen allows only one sync wait on its weight-load slot)
    xs = detection[:, 0].astype(np.int64)
    ys = detection[:, 1].astype(np.int64)
    xpad2 = np.pad(x0, ((0, 0), (1, 1), (1, 1)))
    pcw = np.zeros((1280, K + NPARAM), np.float32)
    for k in range(K):
        pcw[:CONTRACT, k] = xpad2[:, ys[k]:ys[k] + 3, xs[k]:xs[k] + 3].ravel()
    pcw[CONTRACT, :K] = 1.0
    pcw[:, K:] = cwT

    # detection centers replicated 8x along partitions, per 16-instance group:
    # detfan[kl*8+o, g] = 4*x_{16g+kl} (cols 0..7), 4*y (cols 7..14)
    det4 = detection.astype(np.float32) * STRIDE
    detfan = np.zeros((128, 14), np.float32)
    for g, (k0, gsz) in enumerate(GROUPS):
        for kl in range(gsz):
            detfan[kl * 8:kl * 8 + 8, g] = det4[k0 + kl, 0]
            detfan[kl * 8:kl * 8 + 8, 7 + g] = det4[k0 + kl, 1]

    onesbd = np.zeros((128, 16), np.float32)
    for kl in range(16):
        onesbd[kl * 8:kl * 8 + 8, kl] = 1.0

    projT = np.ascontiguousarray(proj_w.T).astype(cdt_np)  # [128, 8]
    projb = proj_b.reshape(8, 1).astype(np.float32)

    # per-core padded input slices
    xpad_rows = np.zeros((128, H + 2 * HALO, F), np.float32)
    xpad_rows[:, HALO:HALO + H, 1:161] = x0
    xpad_rows = xpad_rows.astype(cdt_np)

    shared = dict(twT=twT, pcw=pcw.astype(cdt_np), detfan=detfan,
                  onesbd=onesbd.astype(cdt_np), projT=projT, projb=projb)

    in_maps = []
    for c in range(NCORES):
        xin = np.ascontiguousarray(xpad_rows[:, ROWS * c:ROWS * c + RIN, :])

        # bnv[ch, i, region, 0/1] = inv/shift; zeroed for out-of-image regions
        bnv = np.zeros((128, 4, 3, 2), np.float32)
        for i in range(4):
            bnv[:, i, 1, 0] = inv[i]
            bnv[:, i, 1, 1] = shift[i]
            if c != 0:
                bnv[:, i, 0, 0] = inv[i]
                bnv[:, i, 0, 1] = shift[i]
            if c != NCORES - 1:
                bnv[:, i, 2, 0] = inv[i]
                bnv[:, i, 2, 1] = shift[i]

        grid = np.zeros((2, ROWS, F), np.float32)
        gxrow = -(np.arange(W, dtype=np.float32) * STRIDE + STRIDE // 2)
        gyv = -(np.arange(ROWS * c, ROWS * c + ROWS, dtype=np.float32) * STRIDE
                + STRIDE // 2)
        grid[0, :, 1:161] = gxrow[None, :]
        grid[1, :, 1:161] = gyv[:, None]

        in_maps.append(dict(shared, xin=xin,
                            bnv=bnv.reshape(128, 24),
                            grid=grid.reshape(2, ROWS * F).astype(cdt_np)))
    return in_maps

def _build_program(reps=1):
    """Restructured vs the first working version:

    - weight-assembly scatter is ~23 batched multi-dim DMAs (was ~160
      tiny ones costing ~70us of HWDGE/SWDGE issue): the block-diagonal
      bd matrix is filled with 16 kl-indexed DMAs whose (group, o, o')
      dims are all pure partition- or column-strides.
    - conv4 + proj + dynamic head are interleaved per 3-row chunk, with
      conv4 running one chunk ahead and the head's L0/L1/L2 matmuls
      software-pipelined (skew 2) so PE never waits on a relu.
    - the 7 per-group L2 results land in one [112, nn] PSUM tile
      (disjoint 16-partition ranges) so the +b2 bias is ONE op per chunk
      instead of 7 strided 16-partition ops.
    - elementwise is split: Activation = conv BN + proj bias + half the
      relus; DVE = other half + the L2 bias-add + phase-B assembly math.
    """
    from contextlib import ExitStack
    import concourse.bass as bass
    import concourse.tile as tile
    from concourse import bacc, mybir

    f32 = mybir.dt.float32
    cdt = f32 if _mode() == "fp32" else mybir.dt.bfloat16
    Relu = mybir.ActivationFunctionType.Relu
    Ident = mybir.ActivationFunctionType.Identity
    Alu = mybir.AluOpType

    def man_ap(base, rel_off, dims):
        """manual flat-element AP: dims = [[stride, count], ...]"""
        return bass.AP(tensor=base.tensor, offset=base.offset + rel_off,
                       ap=[list(d) for d in dims])

    nc = bacc.Bacc("TRN2", target_bir_lowering=False, debug=False,
                   enable_asserts=False, detect_race_conditions=False)

    xin_d = nc.dram_tensor("xin", [128, RIN, F], cdt, kind="ExternalInput")
    twT_d = nc.dram_tensor("twT", [128, 36 * 128], cdt, kind="ExternalInput")
    bnv_d = nc.dram_tensor("bnv", [128, 24], f32, kind="ExternalInput")
    grid_d = nc.dram_tensor("grid", [2, P3], cdt, kind="ExternalInput")
    pcw_d = nc.dram_tensor("pcw", [1280, K + NPARAM], cdt, kind="ExternalInput")
    detfan_d = nc.dram_tensor("detfan", [128, 14], f32, kind="ExternalInput")
    onesbd_d = nc.dram_tensor("onesbd", [128, 16], cdt, kind="ExternalInput")
    projT_d = nc.dram_tensor("projT", [128, 8], cdt, kind="ExternalInput")
    projb_d = nc.dram_tensor("projb", [8, 1], f32, kind="ExternalInput")
    out_d = nc.dram_tensor("out", [K, ROWS, W], f32, kind="ExternalOutput")
    p2d_d = nc.dram_tensor("p2d", [112, NPARAM], cdt, kind="Internal")

    NC2 = K + NPARAM   # pcw row width (285)

    with tile.TileContext(nc) as tc, ExitStack() as octx:
      for rep in range(reps):
       with ExitStack() as ctx:
        const = ctx.enter_context(tc.tile_pool(name=f"const{rep}", bufs=1))
        prep = ctx.enter_context(tc.tile_pool(name=f"prep{rep}", bufs=1))
        convp = ctx.enter_context(tc.tile_pool(name=f"conv{rep}", bufs=1))

        # ---------- input DMAs (merged to minimize issue cost) ----------
        # first xin/twT chunk ahead of pcw: conv1's first matmuls only need
        # input rows 0..7 and taps 0..8, so the tower can start ~immediately
        # while pcw (for the params matmuls) streams in behind
        pm = ctx.enter_context(tc.tile_pool(name=f"pm{rep}", bufs=1))
        pc_all = pm.tile([128, 10 * NC2], cdt)
        xbuf = convp.tile([128, RIN * F + 2], cdt, tag="xbuf")
        nc.vector.memset(xbuf[:, 0:1], 0.0)
        nc.vector.memset(xbuf[:, 1 + RIN * F:], 0.0)
        tw_all = const.tile([128, 36 * 128], cdt)
        tw_sb = [tw_all[:, t * 128:(t + 1) * 128] for t in range(36)]

        def _xin_rows(r0, r1):
            nc.sync.dma_start(out=xbuf[:, 1 + r0 * F:1 + r1 * F],
                              in_=xin_d[:, r0:r1, :])

        def _tw_dma(t0, t1):
            nc.sync.dma_start(out=tw_all[:, t0 * 128:t1 * 128],
                              in_=twT_d[:, t0 * 128:t1 * 128])

        # startup criticality order: conv chunk 0 needs rows 0..5 + tap 0
        # first, so those land ~1.5us before the rest
        _xin_rows(0, 5)
        _tw_dma(0, 1)
        bnv_sb = const.tile([128, 24], f32)
        nc.sync.dma_start(out=bnv_sb[:], in_=bnv_d[:])
        _tw_dma(1, 9)
        _xin_rows(5, 14)
        _xin_rows(14, 21)
        _xin_rows(21, 28)
        for h in range(5):
            nc.sync.dma_start(
                out=man_ap(pc_all[:], 2 * h * NC2,
                           [[10 * NC2, 128], [NC2, 2], [1, NC2]]),
                in_=man_ap(pcw_d[:], 2 * h * 128 * NC2,
                           [[NC2, 128], [128 * NC2, 2], [1, NC2]]))
        _tw_dma(9, 36)

        hbase = const.tile([10, P3], cdt)
        nc.sync.dma_start(out=hbase[8:10, :], in_=grid_d[:])
        detfan_sb = const.tile([128, 14], f32)
        nc.sync.dma_start(out=detfan_sb[:], in_=detfan_d[:])
        onesbd_sb = const.tile([128, 16], cdt)
        nc.sync.dma_start(out=onesbd_sb[:], in_=onesbd_d[:])
        projT_sb = const.tile([128, 8], cdt)
        nc.sync.dma_start(out=projT_sb[:], in_=projT_d[:])
        projb_sb = const.tile([8, 1], f32)
        nc.sync.dma_start(out=projb_sb[:], in_=projb_d[:])

        # ---------- phase C start: conv layer 1, first two chunks ----------
        # (emitted before the params matmuls so conv1 isn't gated on the
        # pcw DMAs in the in-order PE queue; params land during chunk 1)
        conv_ps = ctx.enter_context(
            tc.tile_pool(name=f"conv_ps{rep}", bufs=2, space="PSUM"))

        # p-state warmup: ~6 dummy matmuls on a zeroed tile keep PE busy
        # from ~0.8us so it reaches full clock before the first real conv
        # matmul (input DMAs land ~4us); their psum slots recycle via the
        # same cps ring the conv uses
        warm = convp.tile([128, 512], cdt, tag="warm")
        nc.vector.memset(warm[:], 0.0)
        for _ in range(8):
            wp = conv_ps.tile([128, 512], f32, tag="cps")
            nc.tensor.matmul(wp[:], lhsT=warm[:, 0:128], rhs=warm[:],
                             start=True, stop=True)

        def _conv_chunk(i, cur, rout, r0, obuf3):
            nr = min(3, rout - r0)
            ps = conv_ps.tile([128, nr * F], f32, tag="cps")
            for t, (ky, kx) in enumerate(
                    (ky, kx) for ky in range(3) for kx in range(3)):
                off = 1 + (r0 + ky) * F + kx - 1
                nc.tensor.matmul(
                    ps[:], lhsT=tw_sb[i * 9 + t],
                    rhs=cur[:, off:off + nr * F],
                    start=(t == 0), stop=(t == 8))
            ps3 = ps[:].rearrange("p (r c) -> p r c", c=F)
            T = 3 - i  # out-of-image candidate rows at top/bottom
            bounds = sorted({0, T, rout - T, rout})
            for rs, re in zip(bounds[:-1], bounds[1:]):
                a, b = max(rs, r0), min(re, r0 + nr)
                if a >= b:
                    continue
                reg = 0 if b <= T else (2 if a >= rout - T else 1)
                sidx = (i * 3 + reg) * 2
                nc.scalar.activation(
                    out=obuf3[:, a:b, 1:161],
                    in_=ps3[:, a - r0:b - r0, 1:161],
                    func=Relu,
                    scale=bnv_sb[:, sidx:sidx + 1],
                    bias=bnv_sb[:, sidx + 1:sidx + 2])

        def _mk_obuf(i, rout):
            obuf = convp.tile([128, rout * F + 2], cdt, tag=f"c{i}")
            obuf3 = obuf[:, 1:1 + rout * F].rearrange("p (r c) -> p r c", c=F)
            nc.vector.memset(obuf[:, 0:1], 0.0)
            nc.vector.memset(obuf[:, 1 + rout * F:], 0.0)
            nc.vector.memset(obuf3[:, :, 0:1], 0.0)
            nc.vector.memset(obuf3[:, :, 161:162], 0.0)
            return obuf, obuf3

        obuf0, obuf03 = _mk_obuf(0, RIN - 2)
        for r0 in range(0, 24, 3):
            _conv_chunk(0, xbuf, RIN - 2, r0, obuf03)

        # ---------- phase A: dynamic params P2[k, j'] = patches @ ctrl -----
        with tc.tile_pool(name=f"pm_ps{rep}", bufs=1, space="PSUM") as pm_ps:
            p2p = pm_ps.tile([K, NPARAM], f32)
            for i in range(10):
                nc.tensor.matmul(p2p[:],
                                 lhsT=pc_all[:, i * NC2:i * NC2 + K],
                                 rhs=pc_all[:, i * NC2 + K:(i + 1) * NC2],
                                 start=(i == 0), stop=(i == 9))
            # 112 rows: 12 zero rows pad group 6 to a rectangular 7x16
            # instance grid so every assembly DMA below is one rectangle
            p2 = prep.tile([112, NPARAM], cdt)
            nc.vector.memset(p2[:], 0.0)
            nc.vector.tensor_copy(p2[0:100, :], p2p[:])
        P2W = NPARAM

        # ---------- phase B: head weight assembly (batched DMAs + DVE) ----
        # The scatters need multi-partition-strided reads of p2, which the
        # SBUF DMA path can't express (partition steps must lead and be
        # single-step) -- bounce p2 through a DRAM scratch instead: DRAM
        # APs are unconstrained, and every scatter OUT side below leads
        # with its partition dim.
        nc.sync.dma_start(out=p2d_d[:], in_=p2[:])

        # lhsT0 [10, 800]: l0[c, k*8+o] = P2[k, c*8+o] -- one 3D DMA
        l0 = prep.tile([10, 8 * K], cdt)
        nc.sync.dma_start(
            out=man_ap(l0[:], 0, [[8 * K, 10], [8, K], [1, 8]]),
            in_=man_ap(p2d_d[:], 0, [[8, 10], [P2W, K], [1, 8]]))

        # block-diagonal L1 weights, all groups in one [128, 896] tile
        # (group g at cols g*128): one DMA per kl covers all 7 groups,
        # traversal (o, g, o') with the partition dim leading on the out
        BDW = 7 * 128
        bdall = prep.tile([128, BDW], cdt)
        nc.gpsimd.memset(bdall[:], 0.0)
        for kl in range(16):
            nc.sync.dma_start(
                out=man_ap(bdall[:], kl * 8 * BDW + kl * 8,
                           [[BDW, 8], [128, 7], [1, 8]]),
                in_=man_ap(p2d_d[:], kl * P2W + 80,
                           [[8, 8], [16 * P2W, 7], [1, 8]]))

        # fan-out block [kl*8+o, g*5+q], q = (w0x, w0y, b0, b1, w2): the
        # interleaved p2 cols 144 + o*5 + q make each kl's fan one 2D DMA
        # (out rows kl*8..kl*8+8 get all 35 (g,q) cols contiguously; zero
        # rows of p2 fill the fake instances)
        FW = 35
        fanstage = prep.tile([128, FW], cdt)
        for kl in range(16):
            nc.sync.dma_start(
                out=man_ap(fanstage[:], kl * 8 * FW,
                           [[FW, 8], [5, 7], [1, 5]]),
                in_=man_ap(p2d_d[:], kl * P2W + 144,
                           [[5, 8], [16 * P2W, 7], [1, 5]]))
        # b2 per instance as a [112, 1] column (partition == instance id)
        b2stage = prep.tile([112, 1], cdt)
        nc.sync.dma_start(
            out=man_ap(b2stage[:], 0, [[1, 112]]),
            in_=man_ap(p2d_d[:], 184, [[P2W, 112]]))

        fans = prep.tile([128, FW], f32)
        nc.vector.tensor_copy(fans[:], fanstage[:])
        # strided [128, 7] views, one col per group for each quantity q
        w0xfan, w0yfan, b0fan, b1fan, w2fan = (
            man_ap(fans[:], q, [[FW, 128], [5, 7]]) for q in range(5))
        b2col = prep.tile([112, 1], f32)
        nc.vector.tensor_copy(b2col[:], b2stage[:])

        # beta0fan = b0 + w0x*4x + w0y*4y  (per-instance bias, layer 0)
        beta0fan = prep.tile([128, 7], f32)
        tmpf = prep.tile([128, 7], f32)
        nc.vector.tensor_mul(beta0fan[:], w0xfan, detfan_sb[:, 0:7])
        nc.vector.tensor_mul(tmpf[:], w0yfan, detfan_sb[:, 7:14])
        nc.vector.tensor_add(beta0fan[:], beta0fan[:], tmpf[:])
        nc.vector.tensor_add(beta0fan[:], beta0fan[:], b0fan)

        # layer-2 weights: [gp, 112] per group, nonzero only in its own 16
        # output columns -- the 7 L2 matmuls then accumulate into ONE
        # [112, nn] psum tile (PE out base-partition must be 0/32/64, so
        # disjoint 16-row output slices are not addressable directly)
        bd2 = []
        for g, (k0, gsz) in enumerate(GROUPS):
            gp = gsz * 8
            bdw2 = prep.tile([gp, 112], cdt, tag=f"bdw2_{g}")
            nc.vector.memset(bdw2[:], 0.0)
            nc.vector.tensor_scalar_mul(bdw2[:, 16 * g:16 * g + 16],
                                        onesbd_sb[0:gp, :],
                                        fans[0:gp, g * 5 + 4:g * 5 + 5])
            bd2.append(bdw2)

        # ---------- phase C rest: conv layers 1..3 ----------
        for r0 in range(24, RIN - 2, 3):
            _conv_chunk(0, xbuf, RIN - 2, r0, obuf03)
        cur = obuf0
        rcur = RIN - 2
        for i in range(1, 3):
            rout = rcur - 2
            obuf, obuf3 = _mk_obuf(i, rout)
            for r0 in range(0, rout, 3):
                _conv_chunk(i, cur, rout, r0, obuf3)
            cur = obuf
            rcur = rout

        # ---------- phase D: conv4 + proj + dynamic head, interleaved ----
        # conv4 runs one chunk ahead of the head; the head's 21 matmuls per
        # chunk are software-pipelined (L1 two groups behind L0, L2 two
        # behind L1) so relus on Act/DVE complete before PE consumes them.
        NCH = (ROWS + 2) // 3          # 7 chunks of <=3 rows
        def _relu_bias(eng, out, in_, bias):
            if eng == "act":
                nc.scalar.activation(out=out, in_=in_, func=Relu, bias=bias)
            else:
                nc.vector.tensor_scalar(out=out, in0=in_, scalar1=bias,
                                        scalar2=0.0, op0=Alu.add, op1=Alu.max)

        with tc.tile_pool(name=f"c4p{rep}", bufs=2) as c4p, \
             tc.tile_pool(name=f"head{rep}", bufs=6) as headp, \
             tc.tile_pool(name=f"outp{rep}", bufs=2) as outp, \
             tc.tile_pool(name=f"pp_ps{rep}", bufs=1, space="PSUM") as pp_ps, \
             tc.tile_pool(name=f"hps0{rep}", bufs=2, space="PSUM") as hps0, \
             tc.tile_pool(name=f"hps1{rep}", bufs=2, space="PSUM") as hps1, \
             tc.tile_pool(name=f"hps2{rep}", bufs=1, space="PSUM") as hps2:

            def _conv4_thunks(ci):
                """12 emission thunks for conv4+BN+proj of chunk ci, to be
                interleaved between the previous head chunk's t-steps so
                PE never waits on a relu."""
                r0 = 3 * ci
                nr = min(3, ROWS - r0)
                nn = nr * F
                n0 = r0 * F
                state = {}

                def _mm(t):
                    def run():
                        if t == 0:
                            cps = conv_ps.tile([128, nn], f32, tag="cps")
                            state["cps"] = cps
                        ky, kx = t // 3, t % 3
                        off = 1 + (r0 + ky) * F + kx - 1
                        nc.tensor.matmul(
                            state["cps"][:], lhsT=tw_sb[27 + t],
                            rhs=cur[:, off:off + nn],
                            start=(t == 0), stop=(t == 8))
                    return run

                def _bn():
                    # layer-4 rows are all in-image: single mid-region
                    # BN+relu (pad cols get junk, masked at out DMA)
                    c4 = c4p.tile([128, nn], cdt, tag="c4")
                    state["c4"] = c4
                    nc.scalar.activation(out=c4[:], in_=state["cps"][:],
                                         func=Relu,
                                         scale=bnv_sb[:, 20:21],
                                         bias=bnv_sb[:, 21:22])

                def _proj():
                    pp = pp_ps.tile([8, nn], f32, tag="pps")
                    nc.tensor.matmul(pp[:], lhsT=projT_sb[:],
                                     rhs=state["c4"][:],
                                     start=True, stop=True)
                    nc.scalar.activation(out=hbase[0:8, n0:n0 + nn],
                                         in_=pp[:], func=Ident,
                                         bias=projb_sb[:, 0:1])

                return [_mm(t) for t in range(9)] + [_bn, _proj]

            def _head_chunk(ci, fill):
                r0 = 3 * ci
                nr = min(3, ROWS - r0)
                nn = nr * F
                n0 = r0 * F
                ps2 = hps2.tile([112, nn], f32, tag="ps2")
                fi = 0
                h1cs, h2cs = {}, {}
                for t in range(11):
                    if fi < len(fill):
                        fill[fi]()
                        fi += 1
                    if t < 7:
                        g = t
                        k0, gsz = GROUPS[g]
                        gp = gsz * 8
                        ps0 = hps0.tile([gp, nn], f32, tag="ps0")
                        nc.tensor.matmul(ps0[:],
                                         lhsT=l0[:, 8 * k0:8 * k0 + gp],
                                         rhs=hbase[:, n0:n0 + nn],
                                         start=True, stop=True)
                        h1c = headp.tile([gp, nn], cdt, tag="h1c")
                        _relu_bias("act" if g % 2 == 0 else "dve",
                                   h1c[:], ps0[:], beta0fan[0:gp, g:g + 1])
                        h1cs[g] = h1c
                    if 2 <= t < 9:
                        g = t - 2
                        k0, gsz = GROUPS[g]
                        gp = gsz * 8
                        ps1 = hps1.tile([gp, nn], f32, tag="ps1")
                        nc.tensor.matmul(ps1[:],
                                         lhsT=bdall[0:gp,
                                                    g * 128:g * 128 + gp],
                                         rhs=h1cs[g][:],
                                         start=True, stop=True)
                        h2c = headp.tile([gp, nn], cdt, tag="h2c")
                        _relu_bias("dve" if g % 2 == 0 else "act",
                                   h2c[:], ps1[:],
                                   fans[0:gp, g * 5 + 3:g * 5 + 4])
                        h2cs[g] = h2c
                    if 4 <= t:
                        g = t - 4
                        k0, gsz = GROUPS[g]
                        gp = gsz * 8
                        nc.tensor.matmul(ps2[:], lhsT=bd2[g][:],
                                         rhs=h2cs[g][:],
                                         start=(g == 0), stop=(g == 6))
                while fi < len(fill):
                    fill[fi]()
                    fi += 1
                outg = outp.tile([112, nn], f32, tag="outg")
                nc.vector.tensor_scalar(out=outg[:], in0=ps2[:],
                                        scalar1=b2col[:, 0:1], scalar2=None,
                                        op0=Alu.add)
                ogv = outg[0:K, :].rearrange("p (r c) -> p r c", c=F)
                nc.sync.dma_start(out=out_d[:, r0:r0 + nr, :],
                                  in_=ogv[:, :, 1:161])

            for f in _conv4_thunks(0):
                f()
            for ci in range(1, NCH + 1):
                fill = _conv4_thunks(ci) if ci < NCH else []
                _head_chunk(ci - 1, fill)
    nc.compile()
    return nc



def _get_program(reps=1):
    key = (_mode(), reps)
    if key not in _CACHE:
        _CACHE[key] = _build_program(reps)
    return _CACHE[key]


def _run(in_maps, trace=False, reps=1, **kwargs):
    from concourse.bass_utils import run_bass_kernel_spmd
    nc = _get_program(reps)
    return run_bass_kernel_spmd(nc, in_maps, core_ids=list(range(NCORES)),
                                trace=trace, **kwargs)


def kernel(**inputs) -> np.ndarray:
    in_maps = _host_prep(inputs)
    res = _run(in_maps)
    out = np.concatenate([res.results[c]["out"] for c in range(NCORES)], axis=1)
    return out.astype(np.float32)

